# revision 1
# baseline (speedup 1.0000x reference)
"""GPTBigCode MQA causal attention block on 8 TRN2 NeuronCores.

Tensor-parallel over heads: each core computes 4 of 32 query heads (the single
KV head is replicated), row-parallel c_proj, partial outputs summed on host.

All heavy matmuls run as float32r (full PE rate for N>=256). Attention scores
are computed transposed ([k_part, q_free]) so softmax denominators come from a
ones-vector matmul and the P@V product needs no transposes. Softmax skips the
max-subtraction: logits have unit variance by construction, exp() cannot
overflow fp32. Causal masking adds -1e30 to the one triangular 128x128 block
per diagonal tile; fully-masked blocks are never computed.
"""

import numpy as np
from contextlib import ExitStack

import concourse.bass as bass
import concourse.tile as tile
from concourse import mybir
from concourse.bass_utils import run_bass_kernel_spmd
from concourse.masks import make_identity

B, S, D = 2, 2048, 4096
H, DH = 32, 128
KV_DIM = DH
NCORES = 8
HC = H // NCORES          # 4 heads per core
DQC = HC * DH             # 512 q-dims per core
T = B * S                 # 4096 tokens
SCALE = DH ** -0.5
P = 128
NKD = D // P              # 32 contraction tiles in model dim
NMT = T // P              # 32 token tiles of 128
E1 = DQC + 2 * KV_DIM     # 768 = per-core QKV output dims
QTILE = 512               # q tile (free dim) in attention
NQJ = S // QTILE          # 4 q-tiles per batch
NKT = S // P              # 16 k tiles per batch

F32 = mybir.dt.float32
R32 = mybir.dt.float32r
ACTF = mybir.ActivationFunctionType
NEG = -1.0e30


def build_program():
    nc = bass.Bass()
    xt = nc.declare_dram_parameter("xt", [D, T], R32, isOutput=False)
    w1 = nc.declare_dram_parameter("w1", [D, E1], R32, isOutput=False)
    b1 = nc.declare_dram_parameter("b1", [1, E1], R32, isOutput=False)
    w2 = nc.declare_dram_parameter("w2", [DQC, D], R32, isOutput=False)
    b2 = nc.declare_dram_parameter("b2", [P, D // P], F32, isOutput=False)
    onesp = nc.declare_dram_parameter("ones", [1, P], R32, isOutput=False)
    maskp = nc.declare_dram_parameter("mask", [P, P], F32, isOutput=False)
    yt = nc.declare_dram_parameter("yt", [D, T], F32, isOutput=True)
    qt_dram = nc.dram_tensor("qt_scratch", [DQC, T], R32)

    xt3 = xt.rearrange("(kd p) t -> p kd t", p=P)

    with tile.TileContext(nc) as tc:
        with ExitStack() as ctx:
            _body(ctx, tc, nc, xt3, w1, b1, w2, b2, maskp, onesp, yt, qt_dram)
    _legalize_waits(nc)
    return nc


def _legalize_waits(nc, nop_cap=1):
    """walrus's per-instruction sync-wait budget is tiny for matmuls (LDW+MM
    lowering) and DMA pseudo-instructions. Drop redundant same-engine
    self-waits (engines execute in order), then spill excess waits onto
    same-engine NoOps inserted right before the instruction."""
    nocap = (mybir.InstNoOp,)
    f = nc.m.functions[0]
    for bb in f.blocks:
        insts = bb.instructions
        # pass 1: strip same-engine self-waits
        for i in insts:
            si = i.sync_info
            if si is None or not si.on_wait:
                continue
            ename = str(i.engine).split(".")[-1]
            if ename == "SP":
                ename = "Sync"
            kept = [w for w in si.on_wait
                    if w.sync_type != "semaphore"
                    or w.wait_reg is not None
                    or not w.ant_name.split("_")[0] == ename]
            if len(kept) != len(si.on_wait):
                si.on_wait = kept
        # pass 2: spill excess waits onto preceding nops
        idx = 0
        while idx < len(insts):
            i = insts[idx]
            si = i.sync_info
            cap = None if isinstance(i, nocap) else 1
            if cap is not None and si is not None and len(si.on_wait) > cap:
                excess = list(si.on_wait[:-cap])
                si.on_wait = list(si.on_wait[-cap:])
                while excess:
                    chunk, excess = excess[:nop_cap], excess[nop_cap:]
                    nop = mybir.InstNoOp(
                        name=nc.get_next_instruction_name(), ins=[], outs=[])
                    nop.engine = i.engine
                    nop.sync_info = mybir.SyncInfo(on_wait=chunk, on_update=[])
                    nc.register_instruction(nop)
                    insts.insert(idx, nop)
                    idx += 1
            idx += 1


def _body(ctx, tc, nc, xt3, w1, b1, w2, b2, maskp, onesp, yt, qt_dram):
    persist = ctx.enter_context(tc.tile_pool(name="persist", bufs=1))
    kt_sb = persist.tile([P, T], R32)            # K^T [dh, t]
    v_sb = persist.tile([P, NMT, DH], R32)       # V   [t_part, mt, dh]
    ones_row = persist.tile([1, P], R32)         # K=1 stationary for bias aug
    ones_col = persist.tile([P, 1], R32)         # rowsum stationary
    ones_bc = persist.tile([1, P], F32)          # K=1 stationary for bcast (fp32)
    b1row = persist.tile([1, E1], R32)
    b2_sb = persist.tile([P, D // P], F32)
    mask_sb = persist.tile([P, P], F32)          # additive causal mask (0 / -1e30)
    ident = persist.tile([P, P], F32)

    nc.sync.dma_start(out=ones_row[:], in_=onesp[:])
    nc.sync.dma_start(out=ones_col[:], in_=onesp.rearrange("o p -> p o"))
    nc.vector.memset(ones_bc[:], 1.0)
    nc.sync.dma_start(out=b1row[:], in_=b1[:])
    nc.sync.dma_start(out=b2_sb[:], in_=b2[:])
    nc.sync.dma_start(out=mask_sb[:], in_=maskp[:])
    make_identity(nc, ident[:])

    # PSUM pools: 3 + 3 + 2 banks = 8
    ps_mm = ctx.enter_context(tc.tile_pool(name="ps_mm", bufs=3, space="PSUM"))
    ps_acc = ctx.enter_context(tc.tile_pool(name="ps_acc", bufs=3, space="PSUM"))
    ps_aux = ctx.enter_context(tc.tile_pool(name="ps_aux", bufs=2, space="PSUM"))

    # ---------------- Phase A: QKV projection (+bias), layout [t, e] ----------
    with ExitStack() as actx:
        w1_pool = actx.enter_context(tc.tile_pool(name="w1", bufs=1))
        xt_pool = actx.enter_context(tc.tile_pool(name="xtp", bufs=48))
        st_pool = actx.enter_context(tc.tile_pool(name="stage", bufs=8))
        qo_pool = actx.enter_context(tc.tile_pool(name="qout", bufs=6))

        w1_tiles = []
        for kd in range(NKD):
            w1_t = w1_pool.tile([P, E1], R32, tag=f"w1_{kd}")
            nc.sync.dma_start(out=w1_t[:], in_=w1[kd * P:(kd + 1) * P, :])
            w1_tiles.append(w1_t)

        for mt in range(NMT):
            xt_tiles = []
            for kd in range(NKD):
                xt_k = xt_pool.tile([P, P], R32, tag="xt")
                nc.sync.dma_start(
                    out=xt_k[:],
                    in_=xt3[:, kd, mt * P:(mt + 1) * P])
                xt_tiles.append(xt_k)

            ps0 = ps_mm.tile([P, 384], F32, tag="mm")
            ps1 = ps_mm.tile([P, 384], F32, tag="mm")
            for kd in range(NKD):
                lhs = xt_tiles[kd][:]
                nc.tensor.matmul(ps0[:], lhs, w1_tiles[kd][:, 0:384],
                                 start=(kd == 0), stop=False)
                nc.tensor.matmul(ps1[:], lhs, w1_tiles[kd][:, 384:768],
                                 start=(kd == 0), stop=False)
            # bias via ones-row aug (K=1)
            nc.tensor.matmul(ps0[:], ones_row[:], b1row[:, 0:384],
                             start=False, stop=True)
            nc.tensor.matmul(ps1[:], ones_row[:], b1row[:, 384:768],
                             start=False, stop=True)

            # evict; Q heads 0-3 and K go through PE transpose, V is natural
            for h in range(HC):
                ps_src = ps0 if h < 3 else ps1
                off = (h % 3) * P if h < 3 else 0
                q_st = st_pool.tile([P, P], F32, tag="st")
                nc.scalar.activation(q_st[:], ps_src[:, off:off + P], ACTF.Copy)
                tp = ps_aux.tile([P, P], F32, tag="aux")
                nc.tensor.transpose(tp[:], q_st[:], ident[:])
                qo = qo_pool.tile([P, P], R32, tag="qo")
                nc.scalar.activation(qo[:], tp[:], ACTF.Copy)
                nc.sync.dma_start(
                    out=qt_dram[h * P:(h + 1) * P, mt * P:(mt + 1) * P],
                    in_=qo[:])
            k_st = st_pool.tile([P, P], F32, tag="st")
            nc.scalar.activation(k_st[:], ps1[:, P:2 * P], ACTF.Copy)
            tpk = ps_aux.tile([P, P], F32, tag="aux")
            nc.tensor.transpose(tpk[:], k_st[:], ident[:])
            nc.scalar.activation(kt_sb[:, mt * P:(mt + 1) * P], tpk[:], ACTF.Copy)
            nc.scalar.activation(v_sb[:, mt, :], ps1[:, 2 * P:3 * P], ACTF.Copy)

    # ---------------- Phase B+C: attention + c_proj ---------------------------
    w2_pool = ctx.enter_context(tc.tile_pool(name="w2", bufs=1))
    qt_pool = ctx.enter_context(tc.tile_pool(name="qt", bufs=3))
    p_pool = ctx.enter_context(tc.tile_pool(name="pp", bufs=4))
    at_pool = ctx.enter_context(tc.tile_pool(name="at", bufs=8))
    y_pool = ctx.enter_context(tc.tile_pool(name="yp", bufs=3))
    inv_pool = ctx.enter_context(tc.tile_pool(name="inv", bufs=2))

    w2_tiles = []
    for kh in range(HC):
        w2_t = w2_pool.tile([P, D // P, P], R32, tag=f"w2_{kh}")
        nc.sync.dma_start(out=w2_t[:], in_=w2[kh * P:(kh + 1) * P, :])
        w2_tiles.append(w2_t)

    for b in range(B):
        for j in range(NQJ):
            tb = b * S + j * QTILE
            at_tiles = []
            for h in range(HC):
                qt_t = qt_pool.tile([P, QTILE], R32, tag="qt")
                nc.sync.dma_start(out=qt_t[:],
                                  in_=qt_dram[h * P:(h + 1) * P, tb:tb + QTILE])
                ps_out = ps_acc.tile([P, QTILE], F32, tag="acc")
                ps_den = ps_aux.tile([1, QTILE], F32, tag="aux")
                nk = 4 * j + 4
                for kk in range(nk):
                    r = kk - 4 * j
                    qoff = 0 if r < 0 else P * r
                    c0 = b * S + kk * P
                    p_t = p_pool.tile([P, QTILE], R32, tag="p")
                    ps_s = ps_mm.tile([P, QTILE], F32, tag="mm")
                    nc.tensor.matmul(ps_s[:, qoff:], kt_sb[:, c0:c0 + P],
                                     qt_t[:, qoff:], start=True, stop=True)
                    if r >= 0:
                        nc.vector.tensor_add(ps_s[:, qoff:qoff + P],
                                             ps_s[:, qoff:qoff + P], mask_sb[:])
                    nc.scalar.activation(p_t[:, qoff:], ps_s[:, qoff:],
                                         ACTF.Exp, scale=SCALE)
                    nc.tensor.matmul(ps_out[:, qoff:], v_sb[:, b * NKT + kk, :],
                                     p_t[:, qoff:],
                                     start=(kk == 0), stop=(kk == nk - 1))
                    nc.tensor.matmul(ps_den[:, qoff:], ones_col[:],
                                     p_t[:, qoff:],
                                     start=(kk == 0), stop=(kk == nk - 1))
                inv_t = inv_pool.tile([1, QTILE], F32, tag="inv")
                nc.vector.reciprocal(inv_t[:], ps_den[:])
                ps_b = ps_mm.tile([P, QTILE], F32, tag="mm")
                nc.tensor.matmul(ps_b[:], ones_bc[:], inv_t[:],
                                 start=True, stop=True)
                inv_bc = p_pool.tile([P, QTILE], F32, tag="invbc")
                nc.scalar.activation(inv_bc[:], ps_b[:], ACTF.Copy)
                at_t = at_pool.tile([P, QTILE], R32, tag="at")
                nc.vector.tensor_mul(at_t[:], ps_out[:], inv_bc[:])
                at_tiles.append(at_t)
            for me in range(D // P):
                ps_y = ps_acc.tile([P, QTILE], F32, tag="acc")
                for kh in range(HC):
                    nc.tensor.matmul(ps_y[:], w2_tiles[kh][:, me, :],
                                     at_tiles[kh][:],
                                     start=(kh == 0), stop=(kh == HC - 1))
                y_t = y_pool.tile([P, QTILE], F32, tag="y")
                nc.scalar.activation(y_t[:], ps_y[:], ACTF.Identity,
                                     bias=b2_sb[:, me:me + 1])
                nc.sync.dma_start(out=yt[me * P:(me + 1) * P, tb:tb + QTILE],
                                  in_=y_t[:])


_PROGRAM = None


def _get_program():
    global _PROGRAM
    if _PROGRAM is None:
        _PROGRAM = build_program()
    return _PROGRAM


def make_in_maps(hidden_states, w_qkv, b_qkv, w_proj, b_proj):
    x = np.ascontiguousarray(
        np.asarray(hidden_states, dtype=np.float32).reshape(T, D))
    xt = np.ascontiguousarray(x.T)
    # additive causal mask for the triangular block of diagonal tiles
    ki = np.arange(P)[:, None]
    qj = np.arange(P)[None, :]
    mask = np.where(ki <= qj, 0.0, NEG).astype(np.float32)
    w_qkv = np.asarray(w_qkv, dtype=np.float32)
    b_qkv = np.asarray(b_qkv, dtype=np.float32)
    w_proj = np.asarray(w_proj, dtype=np.float32)
    b_proj = np.asarray(b_proj, dtype=np.float32)
    b2 = np.ascontiguousarray(
        (b_proj / NCORES).reshape(D // P, P).T).astype(np.float32)
    in_maps = []
    for c in range(NCORES):
        qcols = slice(c * DQC, (c + 1) * DQC)
        w1 = np.concatenate([w_qkv[:, qcols], w_qkv[:, D:]], axis=1)
        b1 = np.concatenate([b_qkv[qcols], b_qkv[D:]])[None, :]
        w2 = w_proj[c * DQC:(c + 1) * DQC, :]
        in_maps.append({
            "xt": xt,
            "w1": np.ascontiguousarray(w1),
            "b1": np.ascontiguousarray(b1),
            "w2": np.ascontiguousarray(w2),
            "b2": b2,
            "mask": mask,
            "ones": np.ones((1, P), dtype=np.float32),
        })
    return in_maps


def kernel(hidden_states, w_qkv, b_qkv, w_proj, b_proj):
    nc = _get_program()
    in_maps = make_in_maps(hidden_states, w_qkv, b_qkv, w_proj, b_proj)
    res = run_bass_kernel_spmd(nc, in_maps, list(range(NCORES)))
    yts = [np.asarray(r["yt"], dtype=np.float32) for r in res.results]
    y = np.add.reduce(yts).T
    return np.ascontiguousarray(y.reshape(B, S, D))



# revision 5
# speedup vs baseline: 1.5246x; 1.5246x over previous
"""GPTBigCode MQA causal attention block on 8 TRN2 NeuronCores.

Tensor-parallel over heads: each core computes 4 of 32 query heads (the single
KV head is replicated), row-parallel c_proj, partial outputs summed on host.

v2: all matmul operands are fp16 — same PE stream rate as fp32r (1 cycle/row
at N>=512) but LDWEIGHTS goes through the fast-weight-load path (2 elems/read)
which matters here because this environment compiles with ldw-opt disabled and
every matmul pays its weight load serially. QKV is produced directly
transposed (qkv^T = W1^T X^T with X^T streamed as the moving operand), so Q^T
and K^T come out in attention layout with no PE transposes and no DRAM round
trip; only V needs 4 small transposes per 512-token chunk. One fused pass per
512-token chunk: QKV projection -> 4-head attention -> c_proj, so DMA and the
scalar engine overlap the tensor stream throughout.

Softmax skips max-subtraction (unit-variance logits cannot overflow fp32) and
the denominators come from a ones-column matmul; causal masking adds -1e30 to
the triangular 128x128 block of diagonal tiles; fully masked blocks are never
computed.
"""

import numpy as np
from contextlib import ExitStack

import concourse.bass as bass
import concourse.tile as tile
from concourse import mybir
from concourse.bass_utils import run_bass_kernel_spmd
from concourse.masks import make_identity

B, S, D = 2, 2048, 4096
H, DH = 32, 128
P = 128
NCORES = 8
HC = H // NCORES          # 4 heads per core
DQC = HC * DH             # 512 q-dims per core
T = B * S                 # 4096 tokens
SCALE = DH ** -0.5
NKD = D // P              # 32 contraction tiles in model dim
E1 = DQC + 2 * DH         # 768 per-core QKV output dims
NE = E1 // P              # 6 e-tiles: 0-3 Q heads, 4 K, 5 V
QTILE = 512               # tokens per chunk
NCH = T // QTILE          # 8 chunks
NKT = S // P              # 16 key tiles per batch

F32 = mybir.dt.float32
F16 = mybir.dt.float16
ACTF = mybir.ActivationFunctionType
NEG = -1.0e30


def build_program():
    nc = bass.Bass()
    xt = nc.declare_dram_parameter("xt", [D, T], F16, isOutput=False)
    w1 = nc.declare_dram_parameter("w1", [D, E1], F16, isOutput=False)
    b1 = nc.declare_dram_parameter("b1", [P, NE], F32, isOutput=False)
    w2 = nc.declare_dram_parameter("w2", [DQC, D], F16, isOutput=False)
    onesp = nc.declare_dram_parameter("ones", [P, 1], F16, isOutput=False)
    maskp = nc.declare_dram_parameter("mask", [P, P], F32, isOutput=False)
    yt = nc.declare_dram_parameter("yt", [D, T], F16, isOutput=True)

    xt3 = xt.rearrange("(kd p) t -> p kd t", p=P)
    w13 = w1.rearrange("(kd p) e -> p kd e", p=P)
    w23 = w2.rearrange("(kh p) d -> p kh d", p=P)

    with tile.TileContext(nc) as tc:
        with ExitStack() as ctx:
            _body(ctx, tc, nc, xt3, w13, b1, w23, maskp, onesp, yt)
    _legalize_waits(nc)
    return nc


def _legalize_waits(nc, nop_cap=1):
    """walrus's per-instruction sync-wait budget is tiny for matmuls (LDW+MM
    lowering) and DMA pseudo-instructions. Drop redundant same-engine
    self-waits (engines execute in order), then spill excess waits onto
    same-engine NoOps inserted right before the instruction."""
    nocap = (mybir.InstNoOp,)
    f = nc.m.functions[0]
    for bb in f.blocks:
        insts = bb.instructions
        # pass 1: strip same-engine self-waits
        for i in insts:
            si = i.sync_info
            if si is None or not si.on_wait:
                continue
            ename = str(i.engine).split(".")[-1]
            if ename == "SP":
                ename = "Sync"
            kept = [w for w in si.on_wait
                    if w.sync_type != "semaphore"
                    or w.wait_reg is not None
                    or not w.ant_name.split("_")[0] == ename]
            if len(kept) != len(si.on_wait):
                si.on_wait = kept
        # pass 2: spill excess waits onto preceding nops
        idx = 0
        while idx < len(insts):
            i = insts[idx]
            si = i.sync_info
            cap = None if isinstance(i, nocap) else 1
            if cap is not None and si is not None and len(si.on_wait) > cap:
                excess = list(si.on_wait[:-cap])
                si.on_wait = list(si.on_wait[-cap:])
                while excess:
                    chunk, excess = excess[:nop_cap], excess[nop_cap:]
                    nop = mybir.InstNoOp(
                        name=nc.get_next_instruction_name(), ins=[], outs=[])
                    nop.engine = i.engine
                    nop.sync_info = mybir.SyncInfo(on_wait=chunk, on_update=[])
                    nc.register_instruction(nop)
                    insts.insert(idx, nop)
                    idx += 1
            idx += 1


def _body(ctx, tc, nc, xt3, w13, b1, w23, maskp, onesp, yt):
    persist = ctx.enter_context(tc.tile_pool(name="persist", bufs=1))
    w1_sb = persist.tile([P, NKD, E1], F16)      # qkv weights, lhsT tiles
    w2_sb = persist.tile([P, HC, D], F16)        # c_proj weights, lhsT tiles
    b1_sb = persist.tile([P, NE], F32)
    kt_sb = persist.tile([P, T], F16)            # K^T [dh, t]
    v_sb = persist.tile([P, T // P, DH], F16)    # V   [t_part, mt, dh]
    ones_col = persist.tile([P, 1], F16)         # den stationary (K=P, M=1)
    ones_bc = persist.tile([1, P], F16)          # bcast stationary (K=1, M=P)
    mask_sb = persist.tile([P, P], F32)          # additive causal mask
    ident = persist.tile([P, P], F16)

    nc.sync.dma_start(out=w1_sb[:], in_=w13[:])
    nc.sync.dma_start(out=w2_sb[:], in_=w23[:])
    nc.sync.dma_start(out=b1_sb[:], in_=b1[:])
    nc.sync.dma_start(out=ones_col[:], in_=onesp[:])
    nc.sync.dma_start(out=ones_bc[:], in_=onesp.rearrange("p o -> o p"))
    nc.sync.dma_start(out=mask_sb[:], in_=maskp[:])
    make_identity(nc, ident[:])

    # PSUM pools: 3 + 3 + 2 banks = 8
    ps_mm = ctx.enter_context(tc.tile_pool(name="ps_mm", bufs=3, space="PSUM"))
    ps_acc = ctx.enter_context(tc.tile_pool(name="ps_acc", bufs=3, space="PSUM"))
    ps_aux = ctx.enter_context(tc.tile_pool(name="ps_aux", bufs=2, space="PSUM"))

    xc_pool = ctx.enter_context(tc.tile_pool(name="xc", bufs=2))
    qt_pool = ctx.enter_context(tc.tile_pool(name="qt", bufs=2))
    vv_pool = ctx.enter_context(tc.tile_pool(name="vv", bufs=2))
    p_pool = ctx.enter_context(tc.tile_pool(name="pp", bufs=4))
    at_pool = ctx.enter_context(tc.tile_pool(name="at", bufs=8))
    y_pool = ctx.enter_context(tc.tile_pool(name="yp", bufs=3))
    inv_pool = ctx.enter_context(tc.tile_pool(name="inv", bufs=2))

    for tcn in range(NCH):
        b, j = divmod(tcn, NCH // B)
        t0 = tcn * QTILE

        # ---- QKV projection for this chunk: qkv^T[e, t] = W1^T X^T --------
        xc = xc_pool.tile([P, NKD, QTILE], F16, tag="xc")
        nc.sync.dma_start(out=xc[:], in_=xt3[:, :, t0:t0 + QTILE])
        qt_t = qt_pool.tile([P, HC, QTILE], F16, tag="qt")
        # K and V first so attention's diagonal tiles unblock early
        for e in (4, 5, 0, 1, 2, 3):
            ps = ps_mm.tile([P, QTILE], F32, tag="mm")
            for kd in range(NKD):
                nc.tensor.matmul(ps[:], w1_sb[:, kd, e * P:(e + 1) * P],
                                 xc[:, kd, :],
                                 start=(kd == 0), stop=(kd == NKD - 1))
            if e < HC:
                nc.scalar.activation(qt_t[:, e, :], ps[:], ACTF.Identity,
                                     bias=b1_sb[:, e:e + 1])
            elif e == 4:
                nc.scalar.activation(kt_sb[:, t0:t0 + QTILE], ps[:],
                                     ACTF.Identity, bias=b1_sb[:, e:e + 1])
            else:
                vv = vv_pool.tile([P, QTILE], F16, tag="vv")
                nc.scalar.activation(vv[:], ps[:], ACTF.Identity,
                                     bias=b1_sb[:, e:e + 1])
                for r in range(QTILE // P):
                    tp = ps_aux.tile([P, P], F16, tag="aux")
                    nc.tensor.transpose(tp[:], vv[:, r * P:(r + 1) * P],
                                        ident[:])
                    nc.scalar.activation(v_sb[:, tcn * 4 + r, :], tp[:],
                                         ACTF.Copy)

        # ---- attention for this chunk's 4 heads --------------------------
        at_tiles = []
        for h in range(HC):
            ps_out = ps_acc.tile([P, QTILE], F32, tag="acc")
            ps_den = ps_aux.tile([1, QTILE], F32, tag="aux")
            nk = 4 * j + 4
            for kk in range(nk):
                r = kk - 4 * j
                qoff = 0 if r < 0 else P * r
                c0 = b * S + kk * P
                p_t = p_pool.tile([P, QTILE], F16, tag="p")
                ps_s = ps_mm.tile([P, QTILE], F32, tag="mm")
                nc.tensor.matmul(ps_s[:, qoff:], kt_sb[:, c0:c0 + P],
                                 qt_t[:, h, qoff:], start=True, stop=True)
                if r >= 0:
                    nc.vector.tensor_add(ps_s[:, qoff:qoff + P],
                                         ps_s[:, qoff:qoff + P], mask_sb[:])
                nc.scalar.activation(p_t[:, qoff:], ps_s[:, qoff:],
                                     ACTF.Exp, scale=SCALE)
                nc.tensor.matmul(ps_out[:, qoff:], v_sb[:, b * NKT + kk, :],
                                 p_t[:, qoff:],
                                 start=(kk == 0), stop=(kk == nk - 1))
                nc.tensor.matmul(ps_den[:, qoff:], ones_col[:],
                                 p_t[:, qoff:],
                                 start=(kk == 0), stop=(kk == nk - 1))
            inv_t = inv_pool.tile([1, QTILE], F16, tag="inv")
            with nc.allow_low_precision(reason="1/den in fp16: uniform "
                                        "per-column scale, 0.05% rel err"):
                nc.vector.reciprocal(inv_t[:], ps_den[:])
            ps_b = ps_mm.tile([P, QTILE], F32, tag="mm")
            nc.tensor.matmul(ps_b[:], ones_bc[:], inv_t[:],
                             start=True, stop=True)
            inv_bc = p_pool.tile([P, QTILE], F32, tag="invbc")
            nc.scalar.activation(inv_bc[:], ps_b[:], ACTF.Copy)
            at_t = at_pool.tile([P, QTILE], F16, tag="at")
            nc.vector.tensor_mul(at_t[:], ps_out[:], inv_bc[:])
            at_tiles.append(at_t)

        # ---- c_proj: y^T[dout, t] += W2h^T at_h^T ------------------------
        # First 3 me-tiles run kh 0-2 up front so the tensor engine has
        # at[3]-independent work while head 3's normalization chain drains.
        ps_ys = [ps_acc.tile([P, QTILE], F32, tag="acc", name=f"ps_y{i}")
                 for i in range(3)]
        for me in range(3):
            for kh in range(3):
                nc.tensor.matmul(ps_ys[me][:], w2_sb[:, kh, me * P:(me + 1) * P],
                                 at_tiles[kh][:], start=(kh == 0), stop=False)
        for me in range(D // P):
            if me < 3:
                ps_y = ps_ys[me]
                nc.tensor.matmul(ps_y[:], w2_sb[:, 3, me * P:(me + 1) * P],
                                 at_tiles[3][:], start=False, stop=True)
            else:
                ps_y = ps_acc.tile([P, QTILE], F32, tag="acc")
                for kh in range(HC):
                    nc.tensor.matmul(ps_y[:], w2_sb[:, kh, me * P:(me + 1) * P],
                                     at_tiles[kh][:],
                                     start=(kh == 0), stop=(kh == HC - 1))
            y_t = y_pool.tile([P, QTILE], F16, tag="y")
            nc.scalar.activation(y_t[:], ps_y[:], ACTF.Copy)
            nc.sync.dma_start(out=yt[me * P:(me + 1) * P, t0:t0 + QTILE],
                              in_=y_t[:])


_PROGRAM = None


def _get_program():
    global _PROGRAM
    if _PROGRAM is None:
        _PROGRAM = build_program()
    return _PROGRAM


def make_in_maps(hidden_states, w_qkv, b_qkv, w_proj, b_proj):
    x = np.asarray(hidden_states, dtype=np.float32).reshape(T, D)
    xt = np.ascontiguousarray(x.T.astype(np.float16))
    ki = np.arange(P)[:, None]
    qj = np.arange(P)[None, :]
    mask = np.where(ki <= qj, 0.0, NEG).astype(np.float32)
    w_qkv = np.asarray(w_qkv, dtype=np.float32)
    b_qkv = np.asarray(b_qkv, dtype=np.float32)
    w_proj = np.asarray(w_proj, dtype=np.float32)
    ones = np.ones((P, 1), dtype=np.float16)
    in_maps = []
    for c in range(NCORES):
        qcols = slice(c * DQC, (c + 1) * DQC)
        w1 = np.concatenate([w_qkv[:, qcols], w_qkv[:, D:]], axis=1)
        b1 = np.concatenate([b_qkv[qcols], b_qkv[D:]])
        in_maps.append({
            "xt": xt,
            "w1": np.ascontiguousarray(w1.astype(np.float16)),
            "b1": np.ascontiguousarray(b1.reshape(NE, P).T.astype(np.float32)),
            "w2": np.ascontiguousarray(
                w_proj[c * DQC:(c + 1) * DQC, :].astype(np.float16)),
            "mask": mask,
            "ones": ones,
        })
    return in_maps


def kernel(hidden_states, w_qkv, b_qkv, w_proj, b_proj):
    nc = _get_program()
    in_maps = make_in_maps(hidden_states, w_qkv, b_qkv, w_proj, b_proj)
    res = run_bass_kernel_spmd(nc, in_maps, list(range(NCORES)))
    y = np.zeros((D, T), dtype=np.float32)
    for r in res.results:
        y += np.asarray(r["yt"], dtype=np.float32)
    y = y.T + np.asarray(b_proj, dtype=np.float32)[None, :]
    return np.ascontiguousarray(y.reshape(B, S, D)).astype(np.float32)


# revision 6
# speedup vs baseline: 1.8527x; 1.2152x over previous
"""GPTBigCode MQA causal attention block on 8 TRN2 NeuronCores.

Tensor-parallel over heads: each core computes 4 of 32 query heads (the single
KV head is replicated), row-parallel c_proj, partial outputs summed on host.

v2: all matmul operands are fp16 — same PE stream rate as fp32r (1 cycle/row
at N>=512) but LDWEIGHTS goes through the fast-weight-load path (2 elems/read)
which matters here because this environment compiles with ldw-opt disabled and
every matmul pays its weight load serially. QKV is produced directly
transposed (qkv^T = W1^T X^T with X^T streamed as the moving operand), so Q^T
and K^T come out in attention layout with no PE transposes and no DRAM round
trip; only V needs 4 small transposes per 512-token chunk. One fused pass per
512-token chunk: QKV projection -> 4-head attention -> c_proj, so DMA and the
scalar engine overlap the tensor stream throughout.

Softmax skips max-subtraction (unit-variance logits cannot overflow fp32) and
the denominators come from a ones-column matmul; causal masking adds -1e30 to
the triangular 128x128 block of diagonal tiles; fully masked blocks are never
computed.
"""

import numpy as np
from contextlib import ExitStack

import concourse.bass as bass
import concourse.tile as tile
from concourse import mybir
from concourse.bass_utils import run_bass_kernel_spmd
from concourse.masks import make_identity

B, S, D = 2, 2048, 4096
H, DH = 32, 128
P = 128
NCORES = 8
HC = H // NCORES          # 4 heads per core
DQC = HC * DH             # 512 q-dims per core
T = B * S                 # 4096 tokens
SCALE = DH ** -0.5
NKD = D // P              # 32 contraction tiles in model dim
E1 = DQC + 2 * DH         # 768 per-core QKV output dims
NE = E1 // P              # 6 e-tiles: 0-3 Q heads, 4 K, 5 V
QTILE = 512               # tokens per chunk
NCH = T // QTILE          # 8 chunks
NKT = S // P              # 16 key tiles per batch

F32 = mybir.dt.float32
F16 = mybir.dt.float16
ACTF = mybir.ActivationFunctionType
NEG = -1.0e30


def build_program():
    nc = bass.Bass()
    xt = nc.declare_dram_parameter("xt", [D, T], F16, isOutput=False)
    w1 = nc.declare_dram_parameter("w1", [D, E1], F16, isOutput=False)
    b1 = nc.declare_dram_parameter("b1", [P, NE], F32, isOutput=False)
    w2 = nc.declare_dram_parameter("w2", [DQC, D], F16, isOutput=False)
    maskp = nc.declare_dram_parameter("mask", [P, P], F32, isOutput=False)
    yt = nc.declare_dram_parameter("yt", [D, T], F16, isOutput=True)

    xt3 = xt.rearrange("(kd p) t -> p kd t", p=P)
    w13 = w1.rearrange("(kd p) e -> p kd e", p=P)
    w23 = w2.rearrange("(kh p) d -> p kh d", p=P)

    with tile.TileContext(nc) as tc:
        with ExitStack() as ctx:
            _body(ctx, tc, nc, xt3, w13, b1, w23, maskp, yt)
    _legalize_waits(nc)
    return nc


def _legalize_waits(nc, nop_cap=1):
    """walrus's per-instruction sync-wait budget is tiny for matmuls (LDW+MM
    lowering) and DMA pseudo-instructions. Drop redundant same-engine
    self-waits (engines execute in order), then spill excess waits onto
    same-engine NoOps inserted right before the instruction."""
    nocap = (mybir.InstNoOp,)
    f = nc.m.functions[0]
    for bb in f.blocks:
        insts = bb.instructions
        # pass 1: strip same-engine self-waits
        for i in insts:
            si = i.sync_info
            if si is None or not si.on_wait:
                continue
            ename = str(i.engine).split(".")[-1]
            if ename == "SP":
                ename = "Sync"
            kept = [w for w in si.on_wait
                    if w.sync_type != "semaphore"
                    or w.wait_reg is not None
                    or not w.ant_name.split("_")[0] == ename]
            if len(kept) != len(si.on_wait):
                si.on_wait = kept
        # pass 2: spill excess waits onto preceding nops
        idx = 0
        while idx < len(insts):
            i = insts[idx]
            si = i.sync_info
            cap = None if isinstance(i, nocap) else 1
            if cap is not None and si is not None and len(si.on_wait) > cap:
                excess = list(si.on_wait[:-cap])
                si.on_wait = list(si.on_wait[-cap:])
                while excess:
                    chunk, excess = excess[:nop_cap], excess[nop_cap:]
                    nop = mybir.InstNoOp(
                        name=nc.get_next_instruction_name(), ins=[], outs=[])
                    nop.engine = i.engine
                    nop.sync_info = mybir.SyncInfo(on_wait=chunk, on_update=[])
                    nc.register_instruction(nop)
                    insts.insert(idx, nop)
                    idx += 1
            idx += 1


def _body(ctx, tc, nc, xt3, w13, b1, w23, maskp, yt):
    persist = ctx.enter_context(tc.tile_pool(name="persist", bufs=1))
    w1_sb = persist.tile([P, NKD, E1], F16)      # qkv weights, lhsT tiles
    w2_sb = persist.tile([P, HC, D], F16)        # c_proj weights, lhsT tiles
    b1_sb = persist.tile([P, NE], F32)
    kt_sb = persist.tile([P, T], F16)            # K^T [dh, t]
    v_sb = persist.tile([P, T // P, DH], F16)    # V   [t_part, mt, dh]
    ones_pp = persist.tile([P, P], F16)          # den stationary (K=P, M=P)
    mask_sb = persist.tile([P, P], F32)          # additive causal mask
    ident = persist.tile([P, P], F16)

    nc.sync.dma_start(out=w1_sb[:], in_=w13[:])
    nc.sync.dma_start(out=w2_sb[:], in_=w23[:])
    nc.sync.dma_start(out=b1_sb[:], in_=b1[:])
    nc.vector.memset(ones_pp[:], 1.0)
    nc.sync.dma_start(out=mask_sb[:], in_=maskp[:])
    make_identity(nc, ident[:])

    # PSUM pools: 3 + 3 + 2 banks = 8
    ps_mm = ctx.enter_context(tc.tile_pool(name="ps_mm", bufs=3, space="PSUM"))
    ps_acc = ctx.enter_context(tc.tile_pool(name="ps_acc", bufs=3, space="PSUM"))
    ps_aux = ctx.enter_context(tc.tile_pool(name="ps_aux", bufs=2, space="PSUM"))

    xc_pool = ctx.enter_context(tc.tile_pool(name="xc", bufs=2))
    qt_pool = ctx.enter_context(tc.tile_pool(name="qt", bufs=2))
    vv_pool = ctx.enter_context(tc.tile_pool(name="vv", bufs=2))
    p_pool = ctx.enter_context(tc.tile_pool(name="pp", bufs=4))
    at_pool = ctx.enter_context(tc.tile_pool(name="at", bufs=8))
    y_pool = ctx.enter_context(tc.tile_pool(name="yp", bufs=3))

    for tcn in range(NCH):
        b, j = divmod(tcn, NCH // B)
        t0 = tcn * QTILE

        # ---- QKV projection for this chunk: qkv^T[e, t] = W1^T X^T --------
        xc = xc_pool.tile([P, NKD, QTILE], F16, tag="xc")
        nc.sync.dma_start(out=xc[:], in_=xt3[:, :, t0:t0 + QTILE])
        qt_t = qt_pool.tile([P, HC, QTILE], F16, tag="qt")
        # K and V first so attention's diagonal tiles unblock early
        for e in (4, 5, 0, 1, 2, 3):
            ps = ps_mm.tile([P, QTILE], F32, tag="mm")
            for kd in range(NKD):
                nc.tensor.matmul(ps[:], w1_sb[:, kd, e * P:(e + 1) * P],
                                 xc[:, kd, :],
                                 start=(kd == 0), stop=(kd == NKD - 1))
            if e < HC:
                nc.scalar.activation(qt_t[:, e, :], ps[:], ACTF.Identity,
                                     bias=b1_sb[:, e:e + 1])
            elif e == 4:
                nc.scalar.activation(kt_sb[:, t0:t0 + QTILE], ps[:],
                                     ACTF.Identity, bias=b1_sb[:, e:e + 1])
            else:
                vv = vv_pool.tile([P, QTILE], F16, tag="vv")
                nc.scalar.activation(vv[:], ps[:], ACTF.Identity,
                                     bias=b1_sb[:, e:e + 1])
                for r in range(QTILE // P):
                    tp = ps_aux.tile([P, P], F16, tag="aux")
                    nc.tensor.transpose(tp[:], vv[:, r * P:(r + 1) * P],
                                        ident[:])
                    nc.scalar.activation(v_sb[:, tcn * 4 + r, :], tp[:],
                                         ACTF.Copy)

        # ---- attention for this chunk's 4 heads --------------------------
        at_tiles = []
        for h in range(HC):
            ps_out = ps_acc.tile([P, QTILE], F32, tag="acc")
            ps_den = ps_aux.tile([P, QTILE], F32, tag="aux")
            nk = 4 * j + 4
            for kk in range(nk):
                r = kk - 4 * j
                qoff = 0 if r < 0 else P * r
                c0 = b * S + kk * P
                p_t = p_pool.tile([P, QTILE], F16, tag="p")
                ps_s = ps_mm.tile([P, QTILE], F32, tag="mm")
                nc.tensor.matmul(ps_s[:, qoff:], kt_sb[:, c0:c0 + P],
                                 qt_t[:, h, qoff:], start=True, stop=True)
                if r >= 0:
                    nc.vector.tensor_add(ps_s[:, qoff:qoff + P],
                                         ps_s[:, qoff:qoff + P], mask_sb[:])
                nc.scalar.activation(p_t[:, qoff:], ps_s[:, qoff:],
                                     ACTF.Exp, scale=SCALE)
                nc.tensor.matmul(ps_out[:, qoff:], v_sb[:, b * NKT + kk, :],
                                 p_t[:, qoff:],
                                 start=(kk == 0), stop=(kk == nk - 1))
                nc.tensor.matmul(ps_den[:, qoff:], ones_pp[:],
                                 p_t[:, qoff:],
                                 start=(kk == 0), stop=(kk == nk - 1))
            inv_bc = p_pool.tile([P, QTILE], F32, tag="invbc")
            nc.vector.reciprocal(inv_bc[:], ps_den[:])
            at_t = at_pool.tile([P, QTILE], F16, tag="at")
            nc.vector.tensor_mul(at_t[:], ps_out[:], inv_bc[:])
            at_tiles.append(at_t)

        # ---- c_proj: y^T[dout, t] += W2h^T at_h^T ------------------------
        # First 3 me-tiles run kh 0-2 up front so the tensor engine has
        # at[3]-independent work while head 3's normalization chain drains.
        ps_ys = [ps_acc.tile([P, QTILE], F32, tag="acc", name=f"ps_y{i}")
                 for i in range(3)]
        for me in range(3):
            for kh in range(3):
                nc.tensor.matmul(ps_ys[me][:], w2_sb[:, kh, me * P:(me + 1) * P],
                                 at_tiles[kh][:], start=(kh == 0), stop=False)
        for me in range(D // P):
            if me < 3:
                ps_y = ps_ys[me]
                nc.tensor.matmul(ps_y[:], w2_sb[:, 3, me * P:(me + 1) * P],
                                 at_tiles[3][:], start=False, stop=True)
            else:
                ps_y = ps_acc.tile([P, QTILE], F32, tag="acc")
                for kh in range(HC):
                    nc.tensor.matmul(ps_y[:], w2_sb[:, kh, me * P:(me + 1) * P],
                                     at_tiles[kh][:],
                                     start=(kh == 0), stop=(kh == HC - 1))
            y_t = y_pool.tile([P, QTILE], F16, tag="y")
            nc.scalar.activation(y_t[:], ps_y[:], ACTF.Copy)
            nc.sync.dma_start(out=yt[me * P:(me + 1) * P, t0:t0 + QTILE],
                              in_=y_t[:])


_PROGRAM = None


def _get_program():
    global _PROGRAM
    if _PROGRAM is None:
        _PROGRAM = build_program()
    return _PROGRAM


def make_in_maps(hidden_states, w_qkv, b_qkv, w_proj, b_proj):
    x = np.asarray(hidden_states, dtype=np.float32).reshape(T, D)
    xt = np.ascontiguousarray(x.T.astype(np.float16))
    ki = np.arange(P)[:, None]
    qj = np.arange(P)[None, :]
    mask = np.where(ki <= qj, 0.0, NEG).astype(np.float32)
    w_qkv = np.asarray(w_qkv, dtype=np.float32)
    b_qkv = np.asarray(b_qkv, dtype=np.float32)
    w_proj = np.asarray(w_proj, dtype=np.float32)
    in_maps = []
    for c in range(NCORES):
        qcols = slice(c * DQC, (c + 1) * DQC)
        w1 = np.concatenate([w_qkv[:, qcols], w_qkv[:, D:]], axis=1)
        b1 = np.concatenate([b_qkv[qcols], b_qkv[D:]])
        in_maps.append({
            "xt": xt,
            "w1": np.ascontiguousarray(w1.astype(np.float16)),
            "b1": np.ascontiguousarray(b1.reshape(NE, P).T.astype(np.float32)),
            "w2": np.ascontiguousarray(
                w_proj[c * DQC:(c + 1) * DQC, :].astype(np.float16)),
            "mask": mask,
        })
    return in_maps


def kernel(hidden_states, w_qkv, b_qkv, w_proj, b_proj):
    nc = _get_program()
    in_maps = make_in_maps(hidden_states, w_qkv, b_qkv, w_proj, b_proj)
    res = run_bass_kernel_spmd(nc, in_maps, list(range(NCORES)))
    y = np.zeros((D, T), dtype=np.float32)
    for r in res.results:
        y += np.asarray(r["yt"], dtype=np.float32)
    y = y.T + np.asarray(b_proj, dtype=np.float32)[None, :]
    return np.ascontiguousarray(y.reshape(B, S, D)).astype(np.float32)
